# revision 11
# baseline (speedup 1.0000x reference)
"""Haar DWT edge-magnitude kernel for TRN2 (8 NeuronCores, SPMD).

out = sqrt(hl^2 + lh^2 + hh^2) of the 2x2 Haar HL/LH/HH stencil with
reflect padding on the right/bottom edges.

Math: with R[p] = x[p] - x[p+W] (vertical diff) and P[p] = x[p] + x[p+W]
(vertical sum), for interior columns
    out^2[p] = 0.5*R^2[p] + 0.5*R^2[p+1] + 0.25*(P[p] - P[p+1])^2
and at the last column (w = W-1) the +1 shifts become -1 shifts.

This version computes in fp16 end-to-end (inputs are converted on the
host; tolerance is 2e-2, fp16 keeps ~5e-4): halves HBM traffic and
doubles DVE throughput (2x packed mode).  Weights are folded so only two
Activation passes are needed per chunk:
    qq = Square(q * sqrt(0.5))            -> 0.5*q^2        (Act)
    rr = R*R,  s = rr[p]+rr[p+1]          (raw, DVE/Pool)
    u  = s + qq = 2*out^2
    out = Sqrt(u * 0.5)                                    (Act)

Layout: input (8,64,256,256) is sharded by batch across 8 cores.  Each
core's 64 images are split into 2 row-halves -> 128 partitions; the free
dimension is the flattened (row, col) raster of the half (128*256) plus
one "halo" row appended at the end (row 128 of the image for the top
half; reflect row 254 for the bottom half).  All stencil taps are then
free-dimension shifts (+1, +W) of a single SBUF tile.
"""

import numpy as np

import concourse.bass as bass
from concourse import bacc, mybir, tile
from concourse.bass_utils import run_bass_kernel_spmd

AF = mybir.ActivationFunctionType
OP = mybir.AluOpType
FP16 = mybir.dt.float16
NPF16 = np.float16

B, C, H, W = 8, 64, 256, 256
NCORES = 8
P = 128                   # SBUF partitions: 64 images x 2 halves
RH = H // 2               # rows per half
FREE = RH * W             # 32768 output elements per partition
# Chunk-row schedule: small chunks at the start shorten the pipeline
# ramp; small chunks at the end shorten the serial drain chain
# (q -> qq -> u -> sqrt -> DMA of the final chunk).
CHUNK_SCHED = [4] + [8] * 14 + [4, 4, 2, 2]
assert sum(CHUNK_SCHED) == RH

SQRT_HALF = float(np.sqrt(0.5))


def build_nc(reps: int = 1):
    nc = bacc.Bacc("TRN2", target_bir_lowering=False)
    xd = nc.dram_tensor("x", [P, FREE + W], FP16, kind="ExternalInput")
    od = nc.dram_tensor("out", [P, FREE], FP16, kind="ExternalOutput")

    with tile.TileContext(nc) as tc:
        with (
            tc.tile_pool(name="io", bufs=4) as io_pool,
            tc.tile_pool(name="tmp", bufs=3) as tmp_pool,
        ):
            for _rep in range(reps):
                base = 0
                for k, rows in enumerate(CHUNK_SCHED):
                    F = rows * W
                    # Load F + W elements (chunk rows + one halo row); one
                    # extra column is allocated but never loaded: reads of it
                    # only feed outputs that the w=W-1 fixup overwrites.
                    t = io_pool.tile([P, F + W + 1], FP16, tag="in")
                    nc.sync.dma_start(t[:, 0 : F + W], xd[:, base : base + F + W])

                    # HW-calibrated balance (Pool TT runs at ~0.52 eff, DVE
                    # fp16 gets 2x, Act ~1 elem/cycle): DVE ~4.1 passes,
                    # Act ~2.5 (sqrt + qq + rr on ~half the chunks),
                    # Pool ~1.3 (u + occasionally P).
                    r_t = tmp_pool.tile([P, F + 1], FP16, tag="r")
                    p_t = tmp_pool.tile([P, F + 1], FP16, tag="p")
                    nc.vector.tensor_tensor(
                        r_t[:], t[:, 0 : F + 1], t[:, W : F + W + 1], OP.subtract
                    )
                    p_eng = nc.gpsimd if (k % 4) == 1 else nc.vector
                    p_eng.tensor_tensor(
                        p_t[:], t[:, 0 : F + 1], t[:, W : F + W + 1], OP.add
                    )

                    # rr: weighted square on Act (odd chunks) or raw mult on
                    # DVE (even chunks); the sqrt scale absorbs the factor.
                    rr_on_act = (k % 2) == 1
                    rr = tmp_pool.tile([P, F + 1], FP16, tag="rr")
                    if rr_on_act:
                        # rr = 2*R^2 ; qq = q^2 ; u = 4*out^2
                        nc.scalar.activation(
                            rr[:], r_t[:], AF.Square, scale=float(np.sqrt(2.0))
                        )
                        qq_scale, sqrt_scale = 1.0, 0.25
                    else:
                        # rr = R^2 ; qq = 0.5*q^2 ; u = 2*out^2
                        nc.vector.tensor_tensor(rr[:], r_t[:], r_t[:], OP.mult)
                        qq_scale, sqrt_scale = SQRT_HALF, 0.5

                    # q = P[p] - P[p+1]
                    q = tmp_pool.tile([P, F], FP16, tag="q")
                    nc.vector.tensor_tensor(
                        q[:], p_t[:, 0:F], p_t[:, 1 : F + 1], OP.subtract
                    )

                    # qq (weighted square on Act)
                    qq = tmp_pool.tile([P, F], FP16, tag="qq")
                    nc.scalar.activation(qq[:], q[:], AF.Square, scale=qq_scale)

                    # s = rr[p] + rr[p+1]
                    s = tmp_pool.tile([P, F], FP16, tag="s")
                    nc.vector.tensor_tensor(
                        s[:], rr[:, 0:F], rr[:, 1 : F + 1], OP.add
                    )

                    # u = s + qq
                    u = tmp_pool.tile([P, F], FP16, tag="u")
                    nc.gpsimd.tensor_tensor(u[:], s[:], qq[:], OP.add)

                    # Last-column fixup: at w = W-1 the +1 shifts become -1.
                    # q_fix = -q[w-1] so qq_fix = qq[w-1]; only s needs a
                    # dedicated strided op.
                    sf = tmp_pool.tile([P, rows], FP16, tag="sf")
                    nc.vector.tensor_tensor(
                        sf[:],
                        rr[:, W - 1 : F : W],
                        rr[:, W - 2 : F : W],
                        OP.add,
                    )
                    nc.vector.tensor_tensor(
                        u[:, W - 1 : F : W], sf[:], qq[:, W - 2 : F : W], OP.add
                    )

                    o = io_pool.tile([P, F], FP16, tag="out")
                    nc.scalar.activation(o[:], u[:], AF.Sqrt, scale=sqrt_scale)
                    nc.sync.dma_start(od[:, base : base + F], o[:])
                    base += F
    nc.compile()
    return nc


def shard_input(x: np.ndarray) -> list[np.ndarray]:
    """(B,C,H,W) f32 -> per-core [P, FREE+W] fp16 arrays with halo."""
    xr = np.ascontiguousarray(x).reshape(B * C, 2, RH, W).astype(NPF16)
    shards = []
    per = (B * C) // NCORES
    for i in range(NCORES):
        xc = xr[i * per : (i + 1) * per]          # (64, 2, RH, W)
        main = xc.reshape(P, FREE)
        halo = np.stack([xc[:, 1, 0, :], xc[:, 1, RH - 2, :]], axis=1)
        arr = np.concatenate([main, halo.reshape(P, W)], axis=1)
        shards.append(np.ascontiguousarray(arr))
    return shards


def unshard_output(outs: list[np.ndarray]) -> np.ndarray:
    per = (B * C) // NCORES
    full = np.empty((B * C, H, W), dtype=np.float32)
    for i, o in enumerate(outs):
        full[i * per : (i + 1) * per] = np.asarray(o, dtype=np.float32).reshape(
            per, H, W
        )
    return full.reshape(B, C, H, W)


def kernel(x: np.ndarray) -> np.ndarray:
    nc = build_nc()
    in_maps = [{"x": s} for s in shard_input(x)]
    res = run_bass_kernel_spmd(nc, in_maps, core_ids=list(range(NCORES)))
    return unshard_output([r["out"] for r in res.results])


# revision 12
# speedup vs baseline: 40.6073x; 40.6073x over previous
"""Haar DWT edge-magnitude kernel for TRN2 (8 NeuronCores, SPMD).

out = sqrt(hl^2 + lh^2 + hh^2) of the 2x2 Haar HL/LH/HH stencil with
reflect padding on the right/bottom edges.

Math: with R[p] = x[p] - x[p+W] (vertical diff) and P[p] = x[p] + x[p+W]
(vertical sum), for interior columns
    out^2[p] = 0.5*R^2[p] + 0.5*R^2[p+1] + 0.25*(P[p] - P[p+1])^2
and at the last column (w = W-1) the +1 shifts become -1 shifts.

This version computes in fp16 end-to-end (inputs are converted on the
host; tolerance is 2e-2, fp16 keeps ~5e-4): halves HBM traffic and
doubles DVE throughput (2x packed mode).  Weights are folded so only two
Activation passes are needed per chunk:
    qq = Square(q * sqrt(0.5))            -> 0.5*q^2        (Act)
    rr = R*R,  s = rr[p]+rr[p+1]          (raw, DVE/Pool)
    u  = s + qq = 2*out^2
    out = Sqrt(u * 0.5)                                    (Act)

Layout: input (8,64,256,256) is sharded by batch across 8 cores.  Each
core's 64 images are split into 2 row-halves -> 128 partitions; the free
dimension is the flattened (row, col) raster of the half (128*256) plus
one "halo" row appended at the end (row 128 of the image for the top
half; reflect row 254 for the bottom half).  All stencil taps are then
free-dimension shifts (+1, +W) of a single SBUF tile.
"""

import numpy as np

import concourse.bass as bass
from concourse import bacc, mybir, tile
from concourse.bass_utils import run_bass_kernel_spmd

AF = mybir.ActivationFunctionType
OP = mybir.AluOpType
FP16 = mybir.dt.float16
NPF16 = np.float16

B, C, H, W = 8, 64, 256, 256
NCORES = 8
P = 128                   # SBUF partitions: 64 images x 2 halves
RH = H // 2               # rows per half
FREE = RH * W             # 32768 output elements per partition
# Chunk-row schedule: small chunks at the start shorten the pipeline
# ramp; small chunks at the end shorten the serial drain chain
# (q -> qq -> u -> sqrt -> DMA of the final chunk).
CHUNK_SCHED = [4] + [8] * 14 + [4, 4, 2, 2]
assert sum(CHUNK_SCHED) == RH

SQRT_HALF = float(np.sqrt(0.5))


def build_nc(reps: int = 1):
    nc = bacc.Bacc("TRN2", target_bir_lowering=False)
    xd = nc.dram_tensor("x", [P, FREE + W], FP16, kind="ExternalInput")
    od = nc.dram_tensor("out", [P, FREE], FP16, kind="ExternalOutput")

    with tile.TileContext(nc) as tc:
        with (
            tc.tile_pool(name="io", bufs=4) as io_pool,
            tc.tile_pool(name="tmp", bufs=3) as tmp_pool,
        ):
            for _rep in range(reps):
                base = 0
                for k, rows in enumerate(CHUNK_SCHED):
                    F = rows * W
                    # Load F + W elements (chunk rows + one halo row); one
                    # extra column is allocated but never loaded: reads of it
                    # only feed outputs that the w=W-1 fixup overwrites.
                    t = io_pool.tile([P, F + W + 1], FP16, tag="in")
                    nc.sync.dma_start(t[:, 0 : F + W], xd[:, base : base + F + W])

                    # HW-calibrated balance (Pool TT runs at ~0.52 eff, DVE
                    # fp16 gets 2x, Act ~1 elem/cycle): DVE ~4.1 passes,
                    # Act ~2.5 (sqrt + qq + rr on ~half the chunks),
                    # Pool ~1.3 (u + occasionally P).
                    r_t = tmp_pool.tile([P, F + 1], FP16, tag="r")
                    p_t = tmp_pool.tile([P, F + 1], FP16, tag="p")
                    nc.vector.tensor_tensor(
                        r_t[:], t[:, 0 : F + 1], t[:, W : F + W + 1], OP.subtract
                    )
                    p_eng = nc.vector
                    p_eng.tensor_tensor(
                        p_t[:], t[:, 0 : F + 1], t[:, W : F + W + 1], OP.add
                    )

                    # rr: weighted square on Act (odd chunks) or raw mult on
                    # DVE (even chunks); the sqrt scale absorbs the factor.
                    rr_on_act = True
                    rr = tmp_pool.tile([P, F + 1], FP16, tag="rr")
                    if rr_on_act:
                        # rr = 2*R^2 ; qq = q^2 ; u = 4*out^2
                        nc.scalar.activation(
                            rr[:], r_t[:], AF.Square, scale=float(np.sqrt(2.0))
                        )
                        qq_scale, sqrt_scale = 1.0, 0.25
                    else:
                        # rr = R^2 ; qq = 0.5*q^2 ; u = 2*out^2
                        nc.vector.tensor_tensor(rr[:], r_t[:], r_t[:], OP.mult)
                        qq_scale, sqrt_scale = SQRT_HALF, 0.5

                    # q = P[p] - P[p+1]
                    q = tmp_pool.tile([P, F], FP16, tag="q")
                    nc.vector.tensor_tensor(
                        q[:], p_t[:, 0:F], p_t[:, 1 : F + 1], OP.subtract
                    )

                    # qq (weighted square on Act)
                    qq = tmp_pool.tile([P, F], FP16, tag="qq")
                    nc.scalar.activation(qq[:], q[:], AF.Square, scale=qq_scale)

                    # s = rr[p] + rr[p+1]
                    s = tmp_pool.tile([P, F], FP16, tag="s")
                    nc.vector.tensor_tensor(
                        s[:], rr[:, 0:F], rr[:, 1 : F + 1], OP.add
                    )

                    # u = s + qq
                    u = tmp_pool.tile([P, F], FP16, tag="u")
                    nc.vector.tensor_tensor(u[:], s[:], qq[:], OP.add)

                    # Last-column fixup: at w = W-1 the +1 shifts become -1.
                    # q_fix = -q[w-1] so qq_fix = qq[w-1]; only s needs a
                    # dedicated strided op.
                    sf = tmp_pool.tile([P, rows], FP16, tag="sf")
                    nc.vector.tensor_tensor(
                        sf[:],
                        rr[:, W - 1 : F : W],
                        rr[:, W - 2 : F : W],
                        OP.add,
                    )
                    nc.vector.tensor_tensor(
                        u[:, W - 1 : F : W], sf[:], qq[:, W - 2 : F : W], OP.add
                    )

                    o = io_pool.tile([P, F], FP16, tag="out")
                    nc.scalar.activation(o[:], u[:], AF.Sqrt, scale=sqrt_scale)
                    nc.sync.dma_start(od[:, base : base + F], o[:])
                    base += F
    nc.compile()
    return nc


def shard_input(x: np.ndarray) -> list[np.ndarray]:
    """(B,C,H,W) f32 -> per-core [P, FREE+W] fp16 arrays with halo."""
    xr = np.ascontiguousarray(x).reshape(B * C, 2, RH, W).astype(NPF16)
    shards = []
    per = (B * C) // NCORES
    for i in range(NCORES):
        xc = xr[i * per : (i + 1) * per]          # (64, 2, RH, W)
        main = xc.reshape(P, FREE)
        halo = np.stack([xc[:, 1, 0, :], xc[:, 1, RH - 2, :]], axis=1)
        arr = np.concatenate([main, halo.reshape(P, W)], axis=1)
        shards.append(np.ascontiguousarray(arr))
    return shards


def unshard_output(outs: list[np.ndarray]) -> np.ndarray:
    per = (B * C) // NCORES
    full = np.empty((B * C, H, W), dtype=np.float32)
    for i, o in enumerate(outs):
        full[i * per : (i + 1) * per] = np.asarray(o, dtype=np.float32).reshape(
            per, H, W
        )
    return full.reshape(B, C, H, W)


def kernel(x: np.ndarray) -> np.ndarray:
    nc = build_nc()
    in_maps = [{"x": s} for s in shard_input(x)]
    res = run_bass_kernel_spmd(nc, in_maps, core_ids=list(range(NCORES)))
    return unshard_output([r["out"] for r in res.results])
